# revision 1
# baseline (speedup 1.0000x reference)
"""Trainium2 Bass kernel for the scatter_memory recurrent MemoryBlock problem.

Reference computation (per batch b):
    qid    = (x - 1) % K + 1
    q      = question_emb[qid]                       # [T, EK]
    inter  = tanh(interaction_emb[x])                # [T, EI]
    w      = softmax(q @ key_memory.T)               # [T, C]
    out[t] = value_memory_init + sum_{s<=t} w[s] (x) inter[s]   # [T, C, EI]

Key algebraic restructuring: every per-token quantity depends only on the
token id x[t] in [0, 220].  So the rank-1 update for token value v is
tabulated once:  UTable[v] = softmax(QG[v] @ keyT) (x) tanh(E[v]),
a [221, 4000] table, and

    out[t] = init + sum_v Counts[t, v] * UTable[v]

where Counts[t, v] = |{s <= t : x[s] = v}| is a cumulative one-hot count.
The gather AND the cumsum over time fuse into plain matmuls.  The full
512-step count matrix of one batch is built in one PSUM accumulation
over its four 128-row one-hot blocks:

    CTall[v, 128k + j] += sum_s Onehot_k[s, v] * TRIO[s, j]

where TRIO[s, j] = 1 iff j >= s (triangle for the block's own steps,
then all-ones columns for every later step; block k only touches
tau >= 128k so only the live columns are streamed).  Then
out[t, f] = sum_v CTall[v, t] * UTable[v, f].  The init vector rides
along as a 222nd vocab row whose count is pinned to 1 by a K=1
broadcast matmul.

Precision/speed: fp32 matmuls cost 4 cycles/column on the PE; fp16 cost 1.
All matmul inputs here are fp16-EXACT on the counts side: one-hot /
triangle masks are 0/1 and counts are integers <= 512 (fp16 represents
integers up to 2048 exactly).  UTable is stored as an fp16 hi+lo pair
(hi = fp16(U), lo = fp16(U - hi), ~2^-22 effective mantissa), so each
output chunk is 4 fp16 matmuls (2 vocab halves x hi/lo) accumulated in
fp32 PSUM.  Measured end-to-end error vs the fp32 reference ~1e-6.

Sharding: data-parallel over batch. 32 batches / 8 cores = 4 per core.
Per core output = 4*512*4000*4B = 33.6 MB -> HBM-write bound (~94us at
358 GB/s/core); PE ~108us of fp16 matmuls, DVE/ACT ~50us each, all
overlapped with the output DMA stream (alternated across the SP and
Pool DGE paths so the two descriptor streams overlap).
"""

import numpy as np

# Problem constants (hardcoded per harness contract).
B, T = 32, 512
K = 110
C = 20
EK = 100
EI = 200
V = 2 * K + 1          # 221 token vocabulary
VI = V                 # vocab slot used as the "init" indicator (221)
VP = 224               # padded vocab (221 tokens + 1 init + 2 pad)
F = C * EI             # 4000 flattened (C, EI)
NCORES = 8
BPC = B // NCORES      # batches per core = 4
PB = 128               # timesteps per block (partition dim)
NBLK = T // PB         # blocks per batch = 4
V1 = 128               # vocab rows handled by UTable part 1
V2 = V - V1 + 1        # 94 = 93 vocab rows + 1 init row in part 2
WW = T                  # TRIO window width: TRI(128) | ONES(384)
NQ = F // 1000          # 4 output chunks per block

_CACHE = {}
LO_SPLIT = True   # include the fp16 lo-plane matmuls (full precision)


def _build_program():
    import concourse.bass as bass
    import concourse.tile as tile
    from concourse import bacc, mybir

    f32 = mybir.dt.float32
    f16 = mybir.dt.float16
    AF = mybir.ActivationFunctionType
    OP = mybir.AluOpType

    # Bacc (not plain Bass): its compile() runs move_matmul_waits_to_ldweights
    # + generate_event_semaphores, which split multi-sem waits to satisfy the
    # TRN2 one-wait-per-instruction constraint.
    nc = bacc.Bacc("TRN2")


    # ---- DRAM parameters ---------------------------------------------------
    # bconst = TRIO [128,512] | iotar [128,224]                     (fp16)
    d_bconst = nc.dram_tensor("bconst", [PB, WW + VP], f16, kind="ExternalInput")
    # qkcat = qgt [100,224] | keyt [100,20] | indcol [100,1]        (f32)
    d_qkcat = nc.dram_tensor("qkcat", [EK, VP + C + 1], f32, kind="ExternalInput")
    d_inter = nc.dram_tensor("interemb", [V, EI], f32, kind="ExternalInput")
    d_xc = nc.dram_tensor("xcols", [PB, BPC * NBLK], f32, kind="ExternalInput")
    d_inithi = nc.dram_tensor("inithi", [BPC, F], f16, kind="ExternalInput")
    d_initlo = nc.dram_tensor("initlo", [BPC, F], f16, kind="ExternalInput")
    d_out = nc.dram_tensor("out", [BPC * T, F], f32, kind="ExternalOutput")

    with tile.TileContext(nc) as tc:
        with (
            tc.tile_pool(name="const", bufs=1) as constp,
            tc.tile_pool(name="ut", bufs=1) as utp,
            tc.tile_pool(name="rpool", bufs=5) as rp,
            tc.tile_pool(name="ctsbp", bufs=2) as ctsbp,
            tc.tile_pool(name="stagep", bufs=3) as stagep,
            tc.tile_pool(name="ctps", bufs=2, space=bass.MemorySpace.PSUM) as ctpsp,
            tc.tile_pool(name="bigps", bufs=4, space=bass.MemorySpace.PSUM) as bigpsp,
        ):
            # ---- load constants -------------------------------------------
            bconst = constp.tile([PB, WW + VP], f16)
            nc.sync.dma_start(bconst[:], d_bconst[:])
            trio = bconst[:, 0:WW]
            iotar = bconst[:, WW : WW + VP]


            qkcat = constp.tile([EK, VP + C + 1], f32)
            nc.sync.dma_start(qkcat[:], d_qkcat[:])
            qgt = qkcat[:, 0:VP]
            keyt = qkcat[:, VP : VP + C]
            indcol = qkcat[:, VP + C : VP + C + 1]   # 1.0 at row 93, else 0

            xf = constp.tile([PB, BPC * NBLK], f32)
            nc.sync.dma_start(xf[:], d_xc[:])
            in1 = constp.tile([V1, EI], f32)
            nc.sync.dma_start(in1[:], d_inter[0:V1, :])
            in2 = constp.tile([V - V1, EI], f32)
            nc.sync.dma_start(in2[:], d_inter[V1:V, :])

            # ---- per-vocab softmax weights (fp32, tiny) -------------------
            lg1 = ctpsp.tile([PB, C], f32, tag="ct1")
            nc.tensor.matmul(lg1[:], qgt[:, 0:V1], keyt[:], start=True, stop=True)
            lg2 = ctpsp.tile([V - V1, C], f32, tag="ct2")
            nc.tensor.matmul(lg2[:], qgt[:, V1:V], keyt[:], start=True, stop=True)

            # softmax without max-subtraction: |logits| <= ~45 here, far
            # inside the fp32 exp range, and exp(l)/sum(exp(l)) is exact.
            w1 = constp.tile([PB, C], f32)
            w2 = constp.tile([V - V1, C], f32)
            for lg, w, p in ((lg1, w1, PB), (lg2, w2, V - V1)):
                sm = constp.tile([p, 1], f32, tag=f"sm{p}")
                nc.scalar.activation(w[:], lg[:], AF.Exp, accum_out=sm[:])
                rc = constp.tile([p, 1], f32, tag=f"rc{p}")
                nc.vector.reciprocal(rc[:], sm[:])
                nc.vector.tensor_scalar_mul(w[:], w[:], rc[:, 0:1])

            # ---- tanh of interaction embeddings ---------------------------
            t1 = constp.tile([V1, EI], f32)
            nc.scalar.activation(t1[:], in1[:], AF.Tanh)
            t2 = constp.tile([V - V1, EI], f32)
            nc.scalar.activation(t2[:], in2[:], AF.Tanh)

            # ---- UTable as fp16 hi/lo pairs, one tile per 1000-col chunk --
            # (per-chunk tiles keep the first blocks' matmuls from waiting
            # on the whole 20-slice table build)
            ut1hi = [utp.tile([V1, 1000], f16, name=f"ut1hi{q}") for q in range(NQ)]
            ut1lo = [utp.tile([V1, 1000], f16, name=f"ut1lo{q}") for q in range(NQ)]
            ut2 = [
                ([utp.tile([V2, 1000], f16, name=f"ut2hi{s}_{q}") for q in range(NQ)],
                 [utp.tile([V2, 1000], f16, name=f"ut2lo{s}_{q}") for q in range(NQ)])
                for s in range(2)
            ]
            nv = V - V1
            # first writer of each set's init row: emit before the vocab-row
            # build/copies so the row DMA doesn't queue behind them
            for b0 in (0, 1):
                uthi0, utlo0 = ut2[b0]
                for q in range(NQ):
                    qs = slice(q * 1000, (q + 1) * 1000)
                    nc.sync.dma_start(
                        uthi0[q][V2 - 1 : V2, :], d_inithi[b0 : b0 + 1, qs]
                    )
                    nc.gpsimd.dma_start(
                        utlo0[q][V2 - 1 : V2, :], d_initlo[b0 : b0 + 1, qs]
                    )
            for q in range(NQ):
                for ci in range(5):
                    c = 5 * q + ci
                    sl = slice(ci * EI, (ci + 1) * EI)
                    # hi = fp16(w_c*tanh) on ACT; lo = (w_c*tanh) - hi on DVE
                    # (walrus only lowers scalar_tensor_tensor on DVE)
                    if q < 2:
                        nc.scalar.mul(ut1hi[q][:, sl], t1[:], w1[:, c : c + 1])
                    else:
                        nc.vector.tensor_scalar(
                            ut1hi[q][:, sl], t1[:], w1[:, c : c + 1], None,
                            op0=OP.mult,
                        )
                    nc.vector.scalar_tensor_tensor(
                        ut1lo[q][:, sl], t1[:], w1[:, c : c + 1],
                        ut1hi[q][:, sl], op0=OP.mult, op1=OP.subtract,
                    )
                    if q < 3:
                        nc.scalar.mul(
                            ut2[0][0][q][0:nv, sl], t2[:], w2[:, c : c + 1]
                        )
                    else:
                        nc.vector.tensor_scalar(
                            ut2[0][0][q][0:nv, sl], t2[:], w2[:, c : c + 1],
                            None, op0=OP.mult,
                        )
                    nc.vector.scalar_tensor_tensor(
                        ut2[0][1][q][0:nv, sl], t2[:], w2[:, c : c + 1],
                        ut2[0][0][q][0:nv, sl],
                        op0=OP.mult, op1=OP.subtract,
                    )
            # second ut2 set: plain copies, off the critical build chain
            for q in range(NQ):
                nc.gpsimd.tensor_copy(ut2[1][0][q][0:nv, :], ut2[0][0][q][0:nv, :])
                nc.gpsimd.tensor_copy(ut2[1][1][q][0:nv, :], ut2[0][1][q][0:nv, :])

            # ---- main loop: 4 batches x (batch-wide counts + 4 blocks) ----
            def counts_phase(b):
                uthi, utlo = ut2[b % 2]
                # per-batch init row (host-split fp16 hi/lo), per chunk;
                # b=0/1 rows were already written before the build
                if b >= 2:
                    for q in range(NQ):
                        qs = slice(q * 1000, (q + 1) * 1000)
                        nc.sync.dma_start(
                            uthi[q][V2 - 1 : V2, :], d_inithi[b : b + 1, qs]
                        )
                        nc.gpsimd.dma_start(
                            utlo[q][V2 - 1 : V2, :], d_initlo[b : b + 1, qs]
                        )

                # one-hot rows for the 4 blocks of this batch
                rs = []
                for k in range(NBLK):
                    j = b * NBLK + k
                    r = rp.tile([PB, VP], f16, tag="r", name=f"r{j}")
                    nc.vector.tensor_scalar(
                        r[:], iotar[:], xf[:, j : j + 1], None, op0=OP.is_equal
                    )
                    rs.append(r)

                # batch-wide counts: CTall[v, tau], tau in [0, 512).
                # Block k only contributes to tau >= 128k, so stream just the
                # live columns of the triangle-then-ones window.
                ct1 = ctpsp.tile([PB, T], f32, tag="ct1", name=f"ct1_{b}")
                ct2 = ctpsp.tile([96, T], f32, tag="ct2", name=f"ct2_{b}")
                for k in range(NBLK):
                    n = T - PB * k
                    nc.tensor.matmul(
                        ct1[:, PB * k : T], rs[k][:, 0:V1], trio[:, 0:n],
                        start=(k == 0), stop=(k == NBLK - 1),
                        skip_group_check=True,
                    )
                for k in range(NBLK):
                    n = T - PB * k
                    nc.tensor.matmul(
                        ct2[:, PB * k : T], rs[k][:, V1:VP], trio[:, 0:n],
                        start=(k == 0), stop=(k == NBLK - 1),
                        skip_group_check=True,
                    )
                ctsb1 = ctsbp.tile([PB, T], f16, tag="ctsb1", name=f"ctsb1_{b}")
                nc.vector.tensor_copy(ctsb1[:], ct1[:])
                # fold the init-indicator (count 1 on row 93, every tau) into
                # the PSUM->SBUF copy as a per-partition bias add
                ctsb2 = ctsbp.tile([96, T], f16, tag="ctsb2", name=f"ctsb2_{b}")
                nc.vector.tensor_scalar_add(ctsb2[:], ct2[:], indcol[0:96, 0:1])
                return ctsb1, ctsb2

            # run counts one batch ahead so the PE has count work to do
            # while the UTable build finishes
            pending = counts_phase(0)
            for b in range(BPC):
                uthi, utlo = ut2[b % 2]
                ctsb1, ctsb2 = pending
                if b + 1 < BPC:
                    pending = counts_phase(b + 1)

                # big matmuls: out[t, f] = sum_v CTall[v, t] * UTable[v, f]
                for k in range(NBLK):
                    j = b * NBLK + k
                    ks = slice(k * PB, (k + 1) * PB)
                    stage = stagep.tile([PB, F], f32, tag="stage")
                    # one 1-bank PSUM tile per 500-col chunk; a pair of
                    # chunks shares each LDWEIGHTS (same stationary counts)
                    for q in range(NQ):
                        c0 = q * 1000
                        pba = bigpsp.tile([PB, 512], f32, name="pba", tag="pb")
                        pbb = bigpsp.tile([PB, 512], f32, name="pbb", tag="pb")
                        pair = ((pba, 0), (pbb, 500))
                        for pb_, c1 in pair:
                            nc.tensor.matmul(
                                pb_[:, 0:500], ctsb1[:, ks],
                                ut1hi[q][:, c1 : c1 + 500],
                                start=True, stop=False,
                            )
                            if LO_SPLIT:
                                nc.tensor.matmul(
                                    pb_[:, 0:500], ctsb1[:, ks],
                                    ut1lo[q][:, c1 : c1 + 500],
                                    start=False, stop=False,
                                )
                        for pb_, c1 in pair:
                            nc.tensor.matmul(
                                pb_[:, 0:500], ctsb2[0:V2, ks],
                                uthi[q][0:V2, c1 : c1 + 500],
                                start=False, stop=not LO_SPLIT,
                            )
                            if LO_SPLIT:
                                nc.tensor.matmul(
                                    pb_[:, 0:500], ctsb2[0:V2, ks],
                                    utlo[q][0:V2, c1 : c1 + 500],
                                    start=False, stop=True,
                                )
                        for pb_, c1 in pair:
                            if (q + c1) % 1000 == 0:
                                nc.vector.tensor_copy(
                                    stage[:, c0 + c1 : c0 + c1 + 500],
                                    pb_[:, 0:500],
                                )
                            else:
                                nc.scalar.copy(
                                    stage[:, c0 + c1 : c0 + c1 + 500],
                                    pb_[:, 0:500],
                                )
                        # stream each 1000-col chunk out as soon as its
                        # copies land; alternate the two DGE paths.  The very
                        # last chunk goes out as two parallel 500-col DMAs to
                        # shorten the drain tail.
                        if j == BPC * NBLK - 1 and q >= NQ - 2:
                            nc.sync.dma_start(
                                d_out[j * PB : (j + 1) * PB, c0 : c0 + 500],
                                stage[:, c0 : c0 + 500],
                            )
                            nc.gpsimd.dma_start(
                                d_out[j * PB : (j + 1) * PB, c0 + 500 : c0 + 1000],
                                stage[:, c0 + 500 : c0 + 1000],
                            )
                        else:
                            dst = d_out[j * PB : (j + 1) * PB, c0 : c0 + 1000]
                            if (j + q) % 2 == 0:
                                nc.sync.dma_start(dst, stage[:, c0 : c0 + 1000])
                            else:
                                nc.gpsimd.dma_start(dst, stage[:, c0 : c0 + 1000])

    nc.compile()
    return nc


def _host_inputs(x, question_emb, interaction_emb, key_memory, value_memory_init):
    """Build the shared constant tensors + per-core shards (all numpy)."""
    x = np.asarray(x).astype(np.int32)
    question_emb = np.asarray(question_emb, dtype=np.float32)
    interaction_emb = np.asarray(interaction_emb, dtype=np.float32)
    key_memory = np.asarray(key_memory, dtype=np.float32)
    value_memory_init = np.asarray(value_memory_init, dtype=np.float32)

    v = np.arange(V, dtype=np.int64)
    qid = (v - 1) % K + 1

    bconst = np.zeros((PB, WW + VP), np.float32)
    # TRIO[s, col] = 1 iff col >= s  (triangle for the block's own 128
    # steps, then all-ones for every later timestep)
    cols = np.arange(WW)[None, :]
    rows = np.arange(PB)[:, None]
    bconst[:, 0:WW] = (cols >= rows).astype(np.float32)
    bconst[:, WW : WW + VP] = np.arange(VP, dtype=np.float32)[None, :]

    qkcat = np.zeros((EK, VP + C + 1), np.float32)
    qkcat[:, :V] = question_emb[qid].T
    qkcat[:, VP : VP + C] = key_memory.T
    qkcat[VI - V1, VP + C] = 1.0       # init indicator at part-2 row 93

    consts = {
        "bconst": bconst.astype(np.float16),
        "qkcat": qkcat,
        "interemb": interaction_emb,
    }

    in_maps = []
    for core in range(NCORES):
        bs = slice(core * BPC, (core + 1) * BPC)
        xc = x[bs]                                  # [BPC, T]
        # xcols[p, b*NBLK + k] = xc[b, k*PB + p]
        xcols = np.ascontiguousarray(
            xc.reshape(BPC, NBLK, PB).transpose(2, 0, 1).reshape(PB, BPC * NBLK)
        ).astype(np.float32)
        initf = value_memory_init[bs].reshape(BPC, F)
        inithi = initf.astype(np.float16)
        initlo = (initf - inithi.astype(np.float32)).astype(np.float16)
        in_maps.append(
            {**consts, "xcols": xcols, "inithi": inithi, "initlo": initlo}
        )
    return in_maps


def kernel(
    x,
    next_question,
    question_emb,
    interaction_emb,
    key_memory,
    value_memory_init,
):
    from concourse.bass_utils import run_bass_kernel_spmd

    if "nc" not in _CACHE:
        _CACHE["nc"] = _build_program()
    nc = _CACHE["nc"]

    in_maps = _host_inputs(
        x, question_emb, interaction_emb, key_memory, value_memory_init
    )
    res = run_bass_kernel_spmd(nc, in_maps, list(range(NCORES)))
    out = np.concatenate(
        [np.asarray(r["out"]).reshape(BPC, T, C, EI) for r in res.results],
        axis=0,
    )
    return out



# revision 2
# speedup vs baseline: 1.6828x; 1.6828x over previous
"""Trainium2 Bass kernel for the scatter_memory recurrent MemoryBlock problem.

Reference computation (per batch b):
    qid    = (x - 1) % K + 1
    q      = question_emb[qid]                       # [T, EK]
    inter  = tanh(interaction_emb[x])                # [T, EI]
    w      = softmax(q @ key_memory.T)               # [T, C]
    out[t] = value_memory_init + sum_{s<=t} w[s] (x) inter[s]   # [T, C, EI]

Every per-token quantity depends only on the token id x[t] in [0, 220], so
the rank-1 update for token value v is a fixed table row
    U[v] = softmax(QG[v] @ keyT) (x) tanh(E[v])          # [221, 4000]
and out[t] = init + sum_v Counts[t, v] * U[v] with Counts the cumulative
one-hot count matrix.  Both U (221 x 4020 flops of softmax/tanh/outer) and
Counts (a cumulative histogram of x) are tiny and data-independent of the
heavy math, so they are precomputed on the host; the device kernel is the
actual heavy contraction
    out[t, f] = sum_v CT_b[v, t] * UT_b[v, f]            # per batch
which is 99.8% of the reference FLOPs, plus the 256 MB output stream.

Layout tricks (per batch, host side):
  * vocab slots are ordered by FIRST USE in that batch, slot 0 = the init
    row (count pinned to 1).  Slots split into group 1 (128 rows) and
    group 2 (96 rows, zero-padded).  Because t < 128 can touch at most
    128 distinct tokens, block 0 of each batch provably has all-zero
    group-2 counts and its second matmul group is skipped (checked on the
    host; a fallback program without the skip is built if the check ever
    fails).
  * counts are integers <= 512, exact in fp16; tables are fp16 (the
    ~2^-11 relative table quantization gives ~1e-3 end-to-end error,
    far inside the 2e-2 gate).
  * the output is written as fp16 and upcast on the host, halving the
    dominant HBM write stream.

Sharding: data-parallel over batch. 32 batches / 8 cores = 4 per core.
Per-core device work: PE = (4+3*2 groups/batch * 4 batches) * 4000 cols
= 112k fp16 columns ~ 47us; output DMA = 64 fp16 chunk writes alternated
over the SP and Pool DGE queues (~25us each); PSUM->SBUF fp16 copies
alternate DVE/ACT (~37/33us).  A few warm-up matmuls on a zeroed tile at
t=0 ramp the PE p-state while the first tables stream in.
"""

import numpy as np

# Problem constants (hardcoded per harness contract).
B, T = 32, 512
K = 110
C = 20
EK = 100
EI = 200
V = 2 * K + 1          # 221 token vocabulary
F = C * EI             # 4000 flattened (C, EI)
NCORES = 8
BPC = B // NCORES      # batches per core = 4
PB = 128               # timesteps per block (partition dim)
NBLK = T // PB         # blocks per batch = 4
S1 = 128               # group-1 slots (slot 0 = init row)
S2 = 96                # group-2 slots (93 max used + padding)
NQ = 4                 # 1000-col output chunks per block

_CACHE = {}


def _build_program(skip0: bool):
    import concourse.bass as bass
    import concourse.tile as tile
    from concourse import bacc, mybir

    f16 = mybir.dt.float16
    f32 = mybir.dt.float32

    nc = bacc.Bacc("TRN2")

    d_ct1 = nc.dram_tensor("ct1", [BPC * S1, T], f16, kind="ExternalInput")
    d_ct2 = nc.dram_tensor("ct2", [BPC * S2, T], f16, kind="ExternalInput")
    d_ut1 = nc.dram_tensor("ut1", [BPC * S1, F], f16, kind="ExternalInput")
    d_ut2 = nc.dram_tensor("ut2", [BPC * S2, F], f16, kind="ExternalInput")
    d_out = nc.dram_tensor("out", [BPC * T, F], f16, kind="ExternalOutput")

    with tile.TileContext(nc) as tc:
        with (
            tc.tile_pool(name="warm", bufs=1) as warmp,
            tc.tile_pool(name="cts", bufs=2) as ctp,
            tc.tile_pool(name="uts", bufs=2) as utp,
            tc.tile_pool(name="stage", bufs=3) as stagep,
            tc.tile_pool(name="wps", bufs=1, space=bass.MemorySpace.PSUM) as wpsp,
            tc.tile_pool(name="mps", bufs=6, space=bass.MemorySpace.PSUM) as mpsp,
        ):
            # ---- PE p-state warm-up while the first tables stream in ------
            wz = warmp.tile([PB, 640], f16)
            nc.gpsimd.memset(wz[:], 0.0)
            wps = wpsp.tile([PB, 512], f32)
            for i in range(7):
                nc.tensor.matmul(
                    wps[:], wz[:, 0:PB], wz[:, PB:640],
                    start=True, stop=True,
                )

            # ---- per-batch input streams (double buffered) ----------------
            def load_batch(b):
                ct1 = ctp.tile([S1, T], f16, tag="ct1", name=f"ct1_{b}")
                ct2 = ctp.tile([S2, T], f16, tag="ct2", name=f"ct2_{b}")
                ut1 = utp.tile([S1, F], f16, tag="ut1", name=f"ut1_{b}")
                ut2 = utp.tile([S2, F], f16, tag="ut2", name=f"ut2_{b}")
                nc.sync.dma_start(ct1[:], d_ct1[b * S1 : (b + 1) * S1, :])
                nc.gpsimd.dma_start(ct2[:], d_ct2[b * S2 : (b + 1) * S2, :])
                # split the big table loads across both DGE queues, in
                # 1000-col pieces so block-0 matmuls can start early
                r1 = slice(b * S1, (b + 1) * S1)
                r2 = slice(b * S2, (b + 1) * S2)
                for q in range(NQ):
                    qs = slice(q * 1000, (q + 1) * 1000)
                    if q % 2 == 0:
                        nc.sync.dma_start(ut1[:, qs], d_ut1[r1, qs])
                        nc.gpsimd.dma_start(ut2[:, qs], d_ut2[r2, qs])
                    else:
                        nc.gpsimd.dma_start(ut1[:, qs], d_ut1[r1, qs])
                        nc.sync.dma_start(ut2[:, qs], d_ut2[r2, qs])
                return ct1, ct2, ut1, ut2

            tiles = load_batch(0)
            for b in range(BPC):
                ct1, ct2, ut1, ut2 = tiles
                if b + 1 < BPC:
                    tiles = load_batch(b + 1)
                for k in range(NBLK):
                    j = b * NBLK + k
                    ks = slice(k * PB, (k + 1) * PB)
                    two_groups = (k > 0) or not skip0
                    stage = stagep.tile([PB, F], f16, tag="stage")
                    for q in range(NQ):
                        c0 = q * 1000
                        pba = mpsp.tile([PB, 512], f32, tag="pb", name="pba")
                        pbb = mpsp.tile([PB, 512], f32, tag="pb", name="pbb")
                        pair = ((pba, c0), (pbb, c0 + 500))
                        for pb_, c in pair:
                            nc.tensor.matmul(
                                pb_[:, 0:500], ct1[:, ks], ut1[:, c : c + 500],
                                start=True, stop=not two_groups,
                            )
                        if two_groups:
                            for pb_, c in pair:
                                nc.tensor.matmul(
                                    pb_[:, 0:500], ct2[:, ks],
                                    ut2[:, c : c + 500],
                                    start=False, stop=True,
                                )
                        for pb_, c in pair:
                            if (q + c) % 1000 == 0:
                                nc.vector.tensor_copy(
                                    stage[:, c : c + 500], pb_[:, 0:500]
                                )
                            else:
                                nc.scalar.copy(
                                    stage[:, c : c + 500], pb_[:, 0:500]
                                )
                        dst = d_out[j * PB : (j + 1) * PB, c0 : c0 + 1000]
                        if (j + q) % 2 == 0:
                            nc.sync.dma_start(dst, stage[:, c0 : c0 + 1000])
                        else:
                            nc.gpsimd.dma_start(dst, stage[:, c0 : c0 + 1000])

    nc.compile()
    return nc


def _host_inputs(x, question_emb, interaction_emb, key_memory, value_memory_init):
    """Host-side precompute: update table, per-batch first-use slot maps,
    cumulative counts, and fp16 shards.  Returns (in_maps, skip0_ok)."""
    x = np.asarray(x).astype(np.int64)
    question_emb = np.asarray(question_emb, dtype=np.float32)
    interaction_emb = np.asarray(interaction_emb, dtype=np.float32)
    key_memory = np.asarray(key_memory, dtype=np.float32)
    value_memory_init = np.asarray(value_memory_init, dtype=np.float32)

    v = np.arange(V, dtype=np.int64)
    qid = (v - 1) % K + 1
    logits = question_emb[qid] @ key_memory.T               # [V, C]
    logits -= logits.max(axis=1, keepdims=True)
    w = np.exp(logits)
    w /= w.sum(axis=1, keepdims=True)                       # [V, C]
    tanhe = np.tanh(interaction_emb)                        # [V, EI]
    utab = (w[:, :, None] * tanhe[:, None, :]).reshape(V, F)
    utab16 = utab.astype(np.float16)                        # [V, F]

    SL = S1 + S2
    skip0_ok = True
    in_maps = []
    for core in range(NCORES):
        ct1 = np.zeros((BPC * S1, T), np.float16)
        ct2 = np.zeros((BPC * S2, T), np.float16)
        ut1 = np.zeros((BPC * S1, F), np.float16)
        ut2 = np.zeros((BPC * S2, F), np.float16)
        for b in range(BPC):
            xb = x[core * BPC + b]                          # [T]
            # order tokens by first use
            _, first_pos = np.unique(xb, return_index=True)
            order = xb[np.sort(first_pos)]                  # [D] token ids
            d = len(order)
            if len(np.unique(xb[:PB])) > S1 - 1:
                skip0_ok = False
            ct = np.zeros((SL, T), np.float16)
            ct[0] = 1.0
            cum = np.cumsum(xb[:, None] == order[None, :], axis=0)  # [T, D]
            ct[1 : 1 + d] = cum.T
            ut = np.zeros((SL, F), np.float16)
            ut[0] = value_memory_init[core * BPC + b].reshape(F)
            ut[1 : 1 + d] = utab16[order]
            ct1[b * S1 : (b + 1) * S1] = ct[0:S1]
            ct2[b * S2 : (b + 1) * S2] = ct[S1:SL]
            ut1[b * S1 : (b + 1) * S1] = ut[0:S1]
            ut2[b * S2 : (b + 1) * S2] = ut[S1:SL]
        in_maps.append({"ct1": ct1, "ct2": ct2, "ut1": ut1, "ut2": ut2})
    return in_maps, skip0_ok


def kernel(
    x,
    next_question,
    question_emb,
    interaction_emb,
    key_memory,
    value_memory_init,
):
    from concourse.bass_utils import run_bass_kernel_spmd

    in_maps, skip0_ok = _host_inputs(
        x, question_emb, interaction_emb, key_memory, value_memory_init
    )
    key = ("nc", skip0_ok)
    if key not in _CACHE:
        _CACHE[key] = _build_program(skip0=skip0_ok)
    nc = _CACHE[key]

    res = run_bass_kernel_spmd(nc, in_maps, list(range(NCORES)))
    out = np.concatenate(
        [
            np.asarray(r["out"]).astype(np.float32).reshape(BPC, T, C, EI)
            for r in res.results
        ],
        axis=0,
    )
    return out


# revision 3
# speedup vs baseline: 2.2727x; 1.3506x over previous
"""Trainium2 Bass kernel for the scatter_memory recurrent MemoryBlock problem.

Reference computation (per batch b):
    qid    = (x - 1) % K + 1
    q      = question_emb[qid]                       # [T, EK]
    inter  = tanh(interaction_emb[x])                # [T, EI]
    w      = softmax(q @ key_memory.T)               # [T, C]
    out[t] = value_memory_init + sum_{s<=t} w[s] (x) inter[s]   # [T, C, EI]

Every per-token quantity depends only on the token id x[t] in [0, 220], so
the rank-1 update for token value v is a fixed table row
    U[v] = softmax(QG[v] @ keyT) (x) tanh(E[v])          # [221, 4000]
and out[t] = init + sum_v Counts[t, v] * U[v] with Counts the cumulative
one-hot count matrix.  Both U (221 x 4020 flops of softmax/tanh/outer) and
Counts (a cumulative histogram of x) are tiny and data-independent of the
heavy math, so they are precomputed on the host; the device kernel is the
actual heavy contraction
    out[t, f] = sum_v CT_b[v, t] * UT_b[v, f]            # per batch
which is 99.8% of the reference FLOPs, plus the 256 MB output stream.

Layout tricks (per batch, host side):
  * vocab slots are ordered by FIRST USE in that batch, slot 0 = the init
    row (count pinned to 1).  Slots split into group 1 (128 rows) and
    group 2 (96 rows, zero-padded).  Because t < 128 can touch at most
    128 distinct tokens, block 0 of each batch provably has all-zero
    group-2 counts and its second matmul group is skipped (checked on the
    host; a fallback program without the skip is built if the check ever
    fails).
  * counts are integers <= 512, exact in fp16; tables are fp16 (the
    ~2^-11 relative table quantization gives ~1e-3 end-to-end error,
    far inside the 2e-2 gate).
  * the output is written as fp16 and upcast on the host, halving the
    dominant HBM write stream.

Sharding: data-parallel over batch. 32 batches / 8 cores = 4 per core.
Per-core device work: PE = (4+3*2 groups/batch * 4 batches) * 4000 cols
= 112k fp16 columns ~ 47us; output DMA = 64 fp16 chunk writes alternated
over the SP and Pool DGE queues (~25us each); PSUM->SBUF fp16 copies
alternate DVE/ACT (~37/33us).  A few warm-up matmuls on a zeroed tile at
t=0 ramp the PE p-state while the first tables stream in.
"""

import numpy as np

# Problem constants (hardcoded per harness contract).
B, T = 32, 512
K = 110
C = 20
EK = 100
EI = 200
V = 2 * K + 1          # 221 token vocabulary
F = C * EI             # 4000 flattened (C, EI)
NCORES = 8
BPC = B // NCORES      # batches per core = 4
PB = 128               # timesteps per block (partition dim)
NBLK = T // PB         # blocks per batch = 4
S1 = 128               # group-1 slots (slot 0 = init row)
S2 = 96                # group-2 slots (93 max used + padding)
NQ = 4                 # 1000-col output chunks per block

_CACHE = {}


def _build_program(skip0: bool):
    import concourse.bass as bass
    import concourse.tile as tile
    from concourse import bacc, mybir

    f16 = mybir.dt.float16
    f32 = mybir.dt.float32

    nc = bacc.Bacc("TRN2")

    d_ct1 = nc.dram_tensor("ct1", [BPC * S1, T], f16, kind="ExternalInput")
    d_ct2 = nc.dram_tensor("ct2", [BPC * S2, T], f16, kind="ExternalInput")
    d_ut1 = nc.dram_tensor("ut1", [BPC * S1, F], f16, kind="ExternalInput")
    d_ut2 = nc.dram_tensor("ut2", [BPC * S2, F], f16, kind="ExternalInput")
    d_out = nc.dram_tensor("out", [BPC * T, F], f16, kind="ExternalOutput")

    with tile.TileContext(nc) as tc:
        with (
            tc.tile_pool(name="warm", bufs=1) as warmp,
            tc.tile_pool(name="cts", bufs=2) as ctp,
            tc.tile_pool(name="uts", bufs=2) as utp,
            tc.tile_pool(name="stage", bufs=3) as stagep,
            tc.tile_pool(name="wps", bufs=1, space=bass.MemorySpace.PSUM) as wpsp,
            tc.tile_pool(name="mps", bufs=6, space=bass.MemorySpace.PSUM) as mpsp,
        ):
            # ---- PE p-state warm-up while the first tables stream in ------
            wz = warmp.tile([PB, 640], f16)
            nc.gpsimd.memset(wz[:], 0.0)
            wps = wpsp.tile([PB, 512], f32)
            for i in range(7):
                nc.tensor.matmul(
                    wps[:], wz[:, 0:PB], wz[:, PB:640],
                    start=True, stop=True,
                )

            # ---- per-batch input streams (double buffered) ----------------
            def load_batch(b):
                ct1 = ctp.tile([S1, T], f16, tag="ct1", name=f"ct1_{b}")
                ct2 = ctp.tile([S2, T], f16, tag="ct2", name=f"ct2_{b}")
                ut1 = utp.tile([S1, F], f16, tag="ut1", name=f"ut1_{b}")
                ut2 = utp.tile([S2, F], f16, tag="ut2", name=f"ut2_{b}")
                nc.sync.dma_start(ct1[:], d_ct1[b * S1 : (b + 1) * S1, :])
                nc.gpsimd.dma_start(ct2[:], d_ct2[b * S2 : (b + 1) * S2, :])
                # split the big table loads across both DGE queues, in
                # 1000-col pieces so block-0 matmuls can start early
                r1 = slice(b * S1, (b + 1) * S1)
                r2 = slice(b * S2, (b + 1) * S2)
                for q in range(NQ):
                    qs = slice(q * 1000, (q + 1) * 1000)
                    if q % 2 == 0:
                        nc.sync.dma_start(ut1[:, qs], d_ut1[r1, qs])
                        nc.gpsimd.dma_start(ut2[:, qs], d_ut2[r2, qs])
                    else:
                        nc.gpsimd.dma_start(ut1[:, qs], d_ut1[r1, qs])
                        nc.sync.dma_start(ut2[:, qs], d_ut2[r2, qs])
                return ct1, ct2, ut1, ut2

            tiles = load_batch(0)
            for b in range(BPC):
                ct1, ct2, ut1, ut2 = tiles
                if b + 1 < BPC:
                    tiles = load_batch(b + 1)
                for k in range(NBLK):
                    j = b * NBLK + k
                    ks = slice(k * PB, (k + 1) * PB)
                    two_groups = (k > 0) or not skip0
                    stage = stagep.tile([PB, F], f16, tag="stage")
                    for q in range(NQ):
                        c0 = q * 1000
                        pba = mpsp.tile([PB, 512], f32, tag="pb", name="pba")
                        pbb = mpsp.tile([PB, 512], f32, tag="pb", name="pbb")
                        pair = ((pba, c0), (pbb, c0 + 500))
                        for pb_, c in pair:
                            nc.tensor.matmul(
                                pb_[:, 0:500], ct1[:, ks], ut1[:, c : c + 500],
                                start=True, stop=not two_groups,
                            )
                        if two_groups:
                            for pb_, c in pair:
                                nc.tensor.matmul(
                                    pb_[:, 0:500], ct2[:, ks],
                                    ut2[:, c : c + 500],
                                    start=False, stop=True,
                                )
                        nc.vector.tensor_copy(
                            stage[:, c0 : c0 + 500], pba[:, 0:500]
                        )
                        nc.scalar.copy(
                            stage[:, c0 + 500 : c0 + 1000], pbb[:, 0:500]
                        )
                        dst = d_out[j * PB : (j + 1) * PB, c0 : c0 + 1000]
                        if (j + q) % 2 == 0:
                            nc.sync.dma_start(dst, stage[:, c0 : c0 + 1000])
                        else:
                            nc.gpsimd.dma_start(dst, stage[:, c0 : c0 + 1000])

    nc.compile()
    return nc


def _host_inputs(x, question_emb, interaction_emb, key_memory, value_memory_init):
    """Host-side precompute: update table, per-batch first-use slot maps,
    cumulative counts, and fp16 shards.  Returns (in_maps, skip0_ok)."""
    x = np.asarray(x).astype(np.int64)
    question_emb = np.asarray(question_emb, dtype=np.float32)
    interaction_emb = np.asarray(interaction_emb, dtype=np.float32)
    key_memory = np.asarray(key_memory, dtype=np.float32)
    value_memory_init = np.asarray(value_memory_init, dtype=np.float32)

    v = np.arange(V, dtype=np.int64)
    qid = (v - 1) % K + 1
    logits = question_emb[qid] @ key_memory.T               # [V, C]
    logits -= logits.max(axis=1, keepdims=True)
    w = np.exp(logits)
    w /= w.sum(axis=1, keepdims=True)                       # [V, C]
    tanhe = np.tanh(interaction_emb)                        # [V, EI]
    utab = (w[:, :, None] * tanhe[:, None, :]).reshape(V, F)
    utab16 = utab.astype(np.float16)                        # [V, F]

    SL = S1 + S2
    skip0_ok = True
    in_maps = []
    for core in range(NCORES):
        ct1 = np.zeros((BPC * S1, T), np.float16)
        ct2 = np.zeros((BPC * S2, T), np.float16)
        ut1 = np.zeros((BPC * S1, F), np.float16)
        ut2 = np.zeros((BPC * S2, F), np.float16)
        for b in range(BPC):
            xb = x[core * BPC + b]                          # [T]
            # order tokens by first use
            _, first_pos = np.unique(xb, return_index=True)
            order = xb[np.sort(first_pos)]                  # [D] token ids
            d = len(order)
            if len(np.unique(xb[:PB])) > S1 - 1:
                skip0_ok = False
            ct = np.zeros((SL, T), np.float16)
            ct[0] = 1.0
            cum = np.cumsum(xb[:, None] == order[None, :], axis=0)  # [T, D]
            ct[1 : 1 + d] = cum.T
            ut = np.zeros((SL, F), np.float16)
            ut[0] = value_memory_init[core * BPC + b].reshape(F)
            ut[1 : 1 + d] = utab16[order]
            ct1[b * S1 : (b + 1) * S1] = ct[0:S1]
            ct2[b * S2 : (b + 1) * S2] = ct[S1:SL]
            ut1[b * S1 : (b + 1) * S1] = ut[0:S1]
            ut2[b * S2 : (b + 1) * S2] = ut[S1:SL]
        in_maps.append({"ct1": ct1, "ct2": ct2, "ut1": ut1, "ut2": ut2})
    return in_maps, skip0_ok


def kernel(
    x,
    next_question,
    question_emb,
    interaction_emb,
    key_memory,
    value_memory_init,
):
    from concourse.bass_utils import run_bass_kernel_spmd

    in_maps, skip0_ok = _host_inputs(
        x, question_emb, interaction_emb, key_memory, value_memory_init
    )
    key = ("nc", skip0_ok)
    if key not in _CACHE:
        _CACHE[key] = _build_program(skip0=skip0_ok)
    nc = _CACHE[key]

    res = run_bass_kernel_spmd(nc, in_maps, list(range(NCORES)))
    out = np.concatenate(
        [
            np.asarray(r["out"]).astype(np.float32).reshape(BPC, T, C, EI)
            for r in res.results
        ],
        axis=0,
    )
    return out


# revision 7
# speedup vs baseline: 2.3318x; 1.0260x over previous
"""Trainium2 Bass kernel for the scatter_memory recurrent MemoryBlock problem.

Reference computation (per batch b):
    qid    = (x - 1) % K + 1
    q      = question_emb[qid]                       # [T, EK]
    inter  = tanh(interaction_emb[x])                # [T, EI]
    w      = softmax(q @ key_memory.T)               # [T, C]
    out[t] = value_memory_init + sum_{s<=t} w[s] (x) inter[s]   # [T, C, EI]

Every per-token quantity depends only on the token id x[t] in [0, 220], so
the rank-1 update for token value v is a fixed table row
    U[v] = softmax(QG[v] @ keyT) (x) tanh(E[v])          # [221, 4000]
and out[t] = init + sum_v Counts[t, v] * U[v] with Counts the cumulative
one-hot count matrix.  Both U (221 x 4020 flops of softmax/tanh/outer) and
Counts (a cumulative histogram of x) are tiny and data-independent of the
heavy math, so they are precomputed on the host; the device kernel is the
actual heavy contraction
    out[t, f] = sum_v CT_b[v, t] * UT_b[v, f]            # per batch
which is 99.8% of the reference FLOPs, plus the 256 MB output stream.

Layout tricks (per batch, host side):
  * vocab slots are ordered by FIRST USE in that batch, slot 0 = the init
    row (count pinned to 1).  Slots split into group 1 (128 rows) and
    group 2 (96 rows, zero-padded).  Because t < 128 can touch at most
    128 distinct tokens, block 0 of each batch provably has all-zero
    group-2 counts and its second matmul group is skipped (checked on the
    host; a fallback program without the skip is built if the check ever
    fails).
  * counts are integers <= 512, exact in fp16; tables are fp16 (the
    ~2^-11 relative table quantization gives ~1e-3 end-to-end error,
    far inside the 2e-2 gate).
  * the output is written as fp16 and upcast on the host, halving the
    dominant HBM write stream.

Sharding: data-parallel over batch. 32 batches / 8 cores = 4 per core.
Per-core device work: PE = (4+3*2 groups/batch * 4 batches) * 4000 cols
= 112k fp16 columns ~ 47us; output DMA = 64 fp16 chunk writes alternated
over the SP and Pool DGE queues (~25us each); PSUM->SBUF fp16 copies
alternate DVE/ACT (~37/33us).  A few warm-up matmuls on a zeroed tile at
t=0 ramp the PE p-state while the first tables stream in.
"""

import numpy as np

# Problem constants (hardcoded per harness contract).
B, T = 32, 512
K = 110
C = 20
EK = 100
EI = 200
V = 2 * K + 1          # 221 token vocabulary
F = C * EI             # 4000 flattened (C, EI)
NCORES = 8
BPC = B // NCORES      # batches per core = 4
PB = 128               # timesteps per block (partition dim)
NBLK = T // PB         # blocks per batch = 4
S1 = 128               # group-1 slots (slot 0 = init row)
S2 = 96                # group-2 slots (93 max used + padding)
NQ = 4                 # 1000-col output chunks per block

_CACHE = {}


def _build_program(skip0: bool):
    import concourse.bass as bass
    import concourse.tile as tile
    from concourse import bacc, mybir

    f16 = mybir.dt.float16
    f32 = mybir.dt.float32

    nc = bacc.Bacc("TRN2")

    d_ct1 = nc.dram_tensor("ct1", [BPC * S1, T], f16, kind="ExternalInput")
    d_ct2 = nc.dram_tensor("ct2", [BPC * S2, T], f16, kind="ExternalInput")
    d_ut1 = nc.dram_tensor("ut1", [BPC * S1, F], f16, kind="ExternalInput")
    d_ut2 = nc.dram_tensor("ut2", [BPC * S2, F], f16, kind="ExternalInput")
    d_out = nc.dram_tensor("out", [BPC * T, F], f16, kind="ExternalOutput")

    with tile.TileContext(nc) as tc:
        with (
            tc.tile_pool(name="warm", bufs=1) as warmp,
            tc.tile_pool(name="cts", bufs=2) as ctp,
            tc.tile_pool(name="uts", bufs=2) as utp,
            tc.tile_pool(name="stage", bufs=3) as stagep,
            tc.tile_pool(name="wps", bufs=1, space=bass.MemorySpace.PSUM) as wpsp,
            tc.tile_pool(name="mps", bufs=7, space=bass.MemorySpace.PSUM) as mpsp,
        ):
            # ---- PE p-state warm-up while the first tables stream in ------
            wz = warmp.tile([PB, 512], f16)
            nc.gpsimd.memset(wz[:], 0.0)
            wps = wpsp.tile([PB, 512], f32)
            for i in range(5):
                nc.tensor.matmul(
                    wps[:], wz[:, 0:PB], wz[:, 0:512],
                    start=True, stop=True,
                )

            # ---- per-batch input streams (double buffered) ----------------
            def load_batch(b):
                ct1 = ctp.tile([S1, T], f16, tag="ct1", name=f"ct1_{b}")
                ct2 = ctp.tile([S2, T], f16, tag="ct2", name=f"ct2_{b}")
                ut1 = utp.tile([S1, F], f16, tag="ut1", name=f"ut1_{b}")
                ut2 = utp.tile([S2, F], f16, tag="ut2", name=f"ut2_{b}")
                r1 = slice(b * S1, (b + 1) * S1)
                r2 = slice(b * S2, (b + 1) * S2)
                # ct1 + the first ut1 piece land first on separate queues so
                # the batch's first (group-1-only) matmul can start ASAP;
                # group-2 inputs follow behind.
                nc.sync.dma_start(ct1[:], d_ct1[r1, :])
                nc.gpsimd.dma_start(ut1[:, 0:500], d_ut1[r1, 0:500])
                nc.sync.dma_start(ut1[:, 500:1000], d_ut1[r1, 500:1000])
                for q in range(1, NQ):
                    qs = slice(q * 1000, (q + 1) * 1000)
                    if q % 2 == 0:
                        nc.sync.dma_start(ut1[:, qs], d_ut1[r1, qs])
                    else:
                        nc.gpsimd.dma_start(ut1[:, qs], d_ut1[r1, qs])
                nc.gpsimd.dma_start(ct2[:], d_ct2[r2, :])
                for q in range(NQ):
                    qs = slice(q * 1000, (q + 1) * 1000)
                    if q % 2 == 0:
                        nc.gpsimd.dma_start(ut2[:, qs], d_ut2[r2, qs])
                    else:
                        nc.sync.dma_start(ut2[:, qs], d_ut2[r2, qs])
                return ct1, ct2, ut1, ut2

            tiles = load_batch(0)
            for b in range(BPC):
                ct1, ct2, ut1, ut2 = tiles
                if b + 1 < BPC:
                    tiles = load_batch(b + 1)
                for k in range(NBLK):
                    j = b * NBLK + k
                    ks = slice(k * PB, (k + 1) * PB)
                    two_groups = (k > 0) or not skip0
                    stage = stagep.tile([PB, F], f16, tag="stage")
                    for q in range(NQ):
                        c0 = q * 1000
                        pba = mpsp.tile([PB, 512], f32, tag="pb", name="pba")
                        pbb = mpsp.tile([PB, 512], f32, tag="pb", name="pbb")
                        pair = ((pba, c0), (pbb, c0 + 500))
                        for pb_, c in pair:
                            nc.tensor.matmul(
                                pb_[:, 0:500], ct1[:, ks], ut1[:, c : c + 500],
                                start=True, stop=not two_groups,
                            )
                        if two_groups:
                            for pb_, c in pair:
                                nc.tensor.matmul(
                                    pb_[:, 0:500], ct2[:, ks],
                                    ut2[:, c : c + 500],
                                    start=False, stop=True,
                                )
                        nc.vector.tensor_copy(
                            stage[:, c0 : c0 + 500], pba[:, 0:500]
                        )
                        nc.scalar.copy(
                            stage[:, c0 + 500 : c0 + 1000], pbb[:, 0:500]
                        )
                        rows = slice(j * PB, (j + 1) * PB)
                        if j == BPC * NBLK - 1 and q == NQ - 1:
                            # shorten the drain tail: the final chunk leaves
                            # as two parallel 500-col DMAs
                            nc.sync.dma_start(
                                d_out[rows, c0 : c0 + 500],
                                stage[:, c0 : c0 + 500],
                            )
                            nc.gpsimd.dma_start(
                                d_out[rows, c0 + 500 : c0 + 1000],
                                stage[:, c0 + 500 : c0 + 1000],
                            )
                        else:
                            dst = d_out[rows, c0 : c0 + 1000]
                            if (j + q) % 2 == 0:
                                nc.sync.dma_start(dst, stage[:, c0 : c0 + 1000])
                            else:
                                nc.gpsimd.dma_start(dst, stage[:, c0 : c0 + 1000])

    nc.compile()
    return nc


def _host_inputs(x, question_emb, interaction_emb, key_memory, value_memory_init):
    """Host-side precompute: update table, per-batch first-use slot maps,
    cumulative counts, and fp16 shards.  Returns (in_maps, skip0_ok)."""
    x = np.asarray(x).astype(np.int64)
    question_emb = np.asarray(question_emb, dtype=np.float32)
    interaction_emb = np.asarray(interaction_emb, dtype=np.float32)
    key_memory = np.asarray(key_memory, dtype=np.float32)
    value_memory_init = np.asarray(value_memory_init, dtype=np.float32)

    v = np.arange(V, dtype=np.int64)
    qid = (v - 1) % K + 1
    logits = question_emb[qid] @ key_memory.T               # [V, C]
    logits -= logits.max(axis=1, keepdims=True)
    w = np.exp(logits)
    w /= w.sum(axis=1, keepdims=True)                       # [V, C]
    tanhe = np.tanh(interaction_emb)                        # [V, EI]
    utab = (w[:, :, None] * tanhe[:, None, :]).reshape(V, F)
    utab16 = utab.astype(np.float16)                        # [V, F]

    SL = S1 + S2
    skip0_ok = True
    in_maps = []
    for core in range(NCORES):
        ct1 = np.zeros((BPC * S1, T), np.float16)
        ct2 = np.zeros((BPC * S2, T), np.float16)
        ut1 = np.zeros((BPC * S1, F), np.float16)
        ut2 = np.zeros((BPC * S2, F), np.float16)
        for b in range(BPC):
            xb = x[core * BPC + b]                          # [T]
            # order tokens by first use
            _, first_pos = np.unique(xb, return_index=True)
            order = xb[np.sort(first_pos)]                  # [D] token ids
            d = len(order)
            if len(np.unique(xb[:PB])) > S1 - 1:
                skip0_ok = False
            ct = np.zeros((SL, T), np.float16)
            ct[0] = 1.0
            cum = np.cumsum(xb[:, None] == order[None, :], axis=0)  # [T, D]
            ct[1 : 1 + d] = cum.T
            ut = np.zeros((SL, F), np.float16)
            ut[0] = value_memory_init[core * BPC + b].reshape(F)
            ut[1 : 1 + d] = utab16[order]
            ct1[b * S1 : (b + 1) * S1] = ct[0:S1]
            ct2[b * S2 : (b + 1) * S2] = ct[S1:SL]
            ut1[b * S1 : (b + 1) * S1] = ut[0:S1]
            ut2[b * S2 : (b + 1) * S2] = ut[S1:SL]
        in_maps.append({"ct1": ct1, "ct2": ct2, "ut1": ut1, "ut2": ut2})
    return in_maps, skip0_ok


def kernel(
    x,
    next_question,
    question_emb,
    interaction_emb,
    key_memory,
    value_memory_init,
):
    from concourse.bass_utils import run_bass_kernel_spmd

    in_maps, skip0_ok = _host_inputs(
        x, question_emb, interaction_emb, key_memory, value_memory_init
    )
    key = ("nc", skip0_ok)
    if key not in _CACHE:
        _CACHE[key] = _build_program(skip0=skip0_ok)
    nc = _CACHE[key]

    res = run_bass_kernel_spmd(nc, in_maps, list(range(NCORES)))
    out = np.concatenate(
        [
            np.asarray(r["out"]).astype(np.float32).reshape(BPC, T, C, EI)
            for r in res.results
        ],
        axis=0,
    )
    return out


# revision 8
# speedup vs baseline: 2.3916x; 1.0256x over previous
"""Trainium2 Bass kernel for the scatter_memory recurrent MemoryBlock problem.

Reference computation (per batch b):
    qid    = (x - 1) % K + 1
    q      = question_emb[qid]                       # [T, EK]
    inter  = tanh(interaction_emb[x])                # [T, EI]
    w      = softmax(q @ key_memory.T)               # [T, C]
    out[t] = value_memory_init + sum_{s<=t} w[s] (x) inter[s]   # [T, C, EI]

Every per-token quantity depends only on the token id x[t] in [0, 220], so
the rank-1 update for token value v is a fixed table row
    U[v] = softmax(QG[v] @ keyT) (x) tanh(E[v])          # [221, 4000]
and out[t] = init + sum_v Counts[t, v] * U[v] with Counts the cumulative
one-hot count matrix.  Both U (221 x 4020 flops of softmax/tanh/outer) and
Counts (a cumulative histogram of x) are tiny and data-independent of the
heavy math, so they are precomputed on the host; the device kernel is the
actual heavy contraction
    out[t, f] = sum_v CT_b[v, t] * UT_b[v, f]            # per batch
which is 99.8% of the reference FLOPs, plus the 256 MB output stream.

Layout tricks (per batch, host side):
  * vocab slots are ordered by FIRST USE in that batch, slot 0 = the init
    row (count pinned to 1).  Slots split into group 1 (128 rows) and
    group 2 (96 rows, zero-padded).  Because t < 128 can touch at most
    128 distinct tokens, block 0 of each batch provably has all-zero
    group-2 counts and its second matmul group is skipped (checked on the
    host; a fallback program without the skip is built if the check ever
    fails).
  * counts are integers <= 512, exact in fp16; tables are fp16 (the
    ~2^-11 relative table quantization gives ~1e-3 end-to-end error,
    far inside the 2e-2 gate).
  * the output is written as fp16 and upcast on the host, halving the
    dominant HBM write stream.

Sharding: data-parallel over batch. 32 batches / 8 cores = 4 per core.
Per-core device work: PE = (4+3*2 groups/batch * 4 batches) * 4000 cols
= 112k fp16 columns ~ 47us; output DMA = 64 fp16 chunk writes alternated
over the SP and Pool DGE queues (~25us each); PSUM->SBUF fp16 copies
alternate DVE/ACT (~37/33us).  A few warm-up matmuls on a zeroed tile at
t=0 ramp the PE p-state while the first tables stream in.
"""

import numpy as np

# Problem constants (hardcoded per harness contract).
B, T = 32, 512
K = 110
C = 20
EK = 100
EI = 200
V = 2 * K + 1          # 221 token vocabulary
F = C * EI             # 4000 flattened (C, EI)
NCORES = 8
BPC = B // NCORES      # batches per core = 4
PB = 128               # timesteps per block (partition dim)
NBLK = T // PB         # blocks per batch = 4
S1 = 128               # group-1 slots (slot 0 = init row)
S2 = 96                # group-2 slots (93 max used + padding)
NQ = 4                 # 1000-col output chunks per block

_CACHE = {}


def _build_program(skip0: bool):
    import concourse.bass as bass
    import concourse.tile as tile
    from concourse import bacc, mybir

    f16 = mybir.dt.float16
    f32 = mybir.dt.float32

    nc = bacc.Bacc("TRN2")

    d_ct1 = nc.dram_tensor("ct1", [BPC * S1, T], f16, kind="ExternalInput")
    d_ct2 = nc.dram_tensor("ct2", [BPC * S2, T], f16, kind="ExternalInput")
    d_ut1 = nc.dram_tensor("ut1", [BPC * S1, F], f16, kind="ExternalInput")
    d_ut2 = nc.dram_tensor("ut2", [BPC * S2, F], f16, kind="ExternalInput")
    d_out = nc.dram_tensor("out", [BPC * T, F], f16, kind="ExternalOutput")

    with tile.TileContext(nc) as tc:
        with (
            tc.tile_pool(name="warm", bufs=1) as warmp,
            tc.tile_pool(name="cts", bufs=2) as ctp,
            tc.tile_pool(name="uts", bufs=2) as utp,
            tc.tile_pool(name="stage", bufs=3) as stagep,
            tc.tile_pool(name="wps", bufs=1, space=bass.MemorySpace.PSUM) as wpsp,
            tc.tile_pool(name="mps", bufs=7, space=bass.MemorySpace.PSUM) as mpsp,
        ):
            # ---- PE p-state warm-up while the first tables stream in ------
            wz = warmp.tile([PB, 512], f16)
            nc.gpsimd.memset(wz[:], 0.0)
            wps = wpsp.tile([PB, 512], f32)
            for i in range(1):
                nc.tensor.matmul(
                    wps[:], wz[:, 0:PB], wz[:, 0:512],
                    start=True, stop=True,
                )

            # ---- per-batch input streams (double buffered) ----------------
            def load_batch(b):
                ct1 = ctp.tile([S1, T], f16, tag="ct1", name=f"ct1_{b}")
                ct2 = ctp.tile([S2, T], f16, tag="ct2", name=f"ct2_{b}")
                ut1 = utp.tile([S1, F], f16, tag="ut1", name=f"ut1_{b}")
                ut2 = utp.tile([S2, F], f16, tag="ut2", name=f"ut2_{b}")
                r1 = slice(b * S1, (b + 1) * S1)
                r2 = slice(b * S2, (b + 1) * S2)
                # ct1 + the first ut1 piece land first on separate queues so
                # the batch's first (group-1-only) matmul can start ASAP;
                # group-2 inputs follow behind.
                nc.sync.dma_start(ct1[:], d_ct1[r1, :])
                nc.gpsimd.dma_start(ut1[:, 0:500], d_ut1[r1, 0:500])
                nc.sync.dma_start(ut1[:, 500:1000], d_ut1[r1, 500:1000])
                for q in range(1, NQ):
                    qs = slice(q * 1000, (q + 1) * 1000)
                    if q % 2 == 0:
                        nc.sync.dma_start(ut1[:, qs], d_ut1[r1, qs])
                    else:
                        nc.gpsimd.dma_start(ut1[:, qs], d_ut1[r1, qs])
                nc.gpsimd.dma_start(ct2[:], d_ct2[r2, :])
                for q in range(NQ):
                    qs = slice(q * 1000, (q + 1) * 1000)
                    if q % 2 == 0:
                        nc.gpsimd.dma_start(ut2[:, qs], d_ut2[r2, qs])
                    else:
                        nc.sync.dma_start(ut2[:, qs], d_ut2[r2, qs])
                return ct1, ct2, ut1, ut2

            tiles = load_batch(0)
            for b in range(BPC):
                ct1, ct2, ut1, ut2 = tiles
                if b + 1 < BPC:
                    tiles = load_batch(b + 1)
                for k in range(NBLK):
                    j = b * NBLK + k
                    ks = slice(k * PB, (k + 1) * PB)
                    two_groups = (k > 0) or not skip0
                    stage = stagep.tile([PB, F], f16, tag="stage")
                    for q in range(NQ):
                        c0 = q * 1000
                        pba = mpsp.tile([PB, 512], f32, tag="pb", name="pba")
                        pbb = mpsp.tile([PB, 512], f32, tag="pb", name="pbb")
                        pair = ((pba, c0), (pbb, c0 + 500))
                        for pb_, c in pair:
                            nc.tensor.matmul(
                                pb_[:, 0:500], ct1[:, ks], ut1[:, c : c + 500],
                                start=True, stop=not two_groups,
                            )
                        if two_groups:
                            for pb_, c in pair:
                                nc.tensor.matmul(
                                    pb_[:, 0:500], ct2[:, ks],
                                    ut2[:, c : c + 500],
                                    start=False, stop=True,
                                )
                        nc.vector.tensor_copy(
                            stage[:, c0 : c0 + 500], pba[:, 0:500]
                        )
                        nc.scalar.copy(
                            stage[:, c0 + 500 : c0 + 1000], pbb[:, 0:500]
                        )
                        rows = slice(j * PB, (j + 1) * PB)
                        if j == BPC * NBLK - 1 and q == NQ - 1:
                            # shorten the drain tail: the final chunk leaves
                            # as two parallel 500-col DMAs
                            nc.sync.dma_start(
                                d_out[rows, c0 : c0 + 500],
                                stage[:, c0 : c0 + 500],
                            )
                            nc.gpsimd.dma_start(
                                d_out[rows, c0 + 500 : c0 + 1000],
                                stage[:, c0 + 500 : c0 + 1000],
                            )
                        else:
                            dst = d_out[rows, c0 : c0 + 1000]
                            if (j + q) % 2 == 0:
                                nc.sync.dma_start(dst, stage[:, c0 : c0 + 1000])
                            else:
                                nc.gpsimd.dma_start(dst, stage[:, c0 : c0 + 1000])

    nc.compile()
    return nc


def _host_inputs(x, question_emb, interaction_emb, key_memory, value_memory_init):
    """Host-side precompute: update table, per-batch first-use slot maps,
    cumulative counts, and fp16 shards.  Returns (in_maps, skip0_ok)."""
    x = np.asarray(x).astype(np.int64)
    question_emb = np.asarray(question_emb, dtype=np.float32)
    interaction_emb = np.asarray(interaction_emb, dtype=np.float32)
    key_memory = np.asarray(key_memory, dtype=np.float32)
    value_memory_init = np.asarray(value_memory_init, dtype=np.float32)

    v = np.arange(V, dtype=np.int64)
    qid = (v - 1) % K + 1
    logits = question_emb[qid] @ key_memory.T               # [V, C]
    logits -= logits.max(axis=1, keepdims=True)
    w = np.exp(logits)
    w /= w.sum(axis=1, keepdims=True)                       # [V, C]
    tanhe = np.tanh(interaction_emb)                        # [V, EI]
    utab = (w[:, :, None] * tanhe[:, None, :]).reshape(V, F)
    utab16 = utab.astype(np.float16)                        # [V, F]

    SL = S1 + S2
    skip0_ok = True
    in_maps = []
    for core in range(NCORES):
        ct1 = np.zeros((BPC * S1, T), np.float16)
        ct2 = np.zeros((BPC * S2, T), np.float16)
        ut1 = np.zeros((BPC * S1, F), np.float16)
        ut2 = np.zeros((BPC * S2, F), np.float16)
        for b in range(BPC):
            xb = x[core * BPC + b]                          # [T]
            # order tokens by first use
            _, first_pos = np.unique(xb, return_index=True)
            order = xb[np.sort(first_pos)]                  # [D] token ids
            d = len(order)
            if len(np.unique(xb[:PB])) > S1 - 1:
                skip0_ok = False
            ct = np.zeros((SL, T), np.float16)
            ct[0] = 1.0
            cum = np.cumsum(xb[:, None] == order[None, :], axis=0)  # [T, D]
            ct[1 : 1 + d] = cum.T
            ut = np.zeros((SL, F), np.float16)
            ut[0] = value_memory_init[core * BPC + b].reshape(F)
            ut[1 : 1 + d] = utab16[order]
            ct1[b * S1 : (b + 1) * S1] = ct[0:S1]
            ct2[b * S2 : (b + 1) * S2] = ct[S1:SL]
            ut1[b * S1 : (b + 1) * S1] = ut[0:S1]
            ut2[b * S2 : (b + 1) * S2] = ut[S1:SL]
        in_maps.append({"ct1": ct1, "ct2": ct2, "ut1": ut1, "ut2": ut2})
    return in_maps, skip0_ok


def kernel(
    x,
    next_question,
    question_emb,
    interaction_emb,
    key_memory,
    value_memory_init,
):
    from concourse.bass_utils import run_bass_kernel_spmd

    in_maps, skip0_ok = _host_inputs(
        x, question_emb, interaction_emb, key_memory, value_memory_init
    )
    key = ("nc", skip0_ok)
    if key not in _CACHE:
        _CACHE[key] = _build_program(skip0=skip0_ok)
    nc = _CACHE[key]

    res = run_bass_kernel_spmd(nc, in_maps, list(range(NCORES)))
    out = np.concatenate(
        [
            np.asarray(r["out"]).astype(np.float32).reshape(BPC, T, C, EI)
            for r in res.results
        ],
        axis=0,
    )
    return out
